# revision 1
# baseline (speedup 1.0000x reference)
"""Trainium2 Bass kernel for out = x @ expm(skew(angles)) + bias.

Strategy:
  - Data-parallel over the batch: x [16384, 512] is split into 8 shards of
    [2048, 512], one per NeuronCore. angles/bias are replicated.
  - Host only does layout: builds A = skew(angles) and -A (pure scatter,
    no FLOPs). All linear algebra runs on-device.
  - On each core the rotation is computed via a degree-6 Taylor series in
    Paterson-Stockmeyer form (3 matmuls of 512^3), exploiting skew-symmetry
    so no on-chip transposes of the 512x512 operands are ever needed:
        A2  = A @ A        (lhsT = -A,  since A^T = -A)
        A3n = -A^3         (lhsT = A2,  since A2 is symmetric)
        B'  = A + A2/5 - A3n/30
        F'  = A3 @ B'      (lhsT = A3n, since A3^T = -A3)
        M   = expm(A) - I = A + A2/2 - A3n/6 + F'/24
    The identity never materializes: out = x + x @ M + bias.
  - Main matmul: per 128-row tile of x, 4 PE transposes build x^T blocks
    (contraction dim must sit on partitions), then 4 accumulating matmuls
    of N=512 plus a rank-1 K=1 matmul that folds in the bias.  The final
    DVE op adds the residual x tile while moving PSUM -> SBUF.

Truncation error of the degree-6 series for this operand norm
(||A||_2 ~ 0.44) is ~4e-8, below fp32 matmul roundoff.
"""

import numpy as np

import concourse.bacc as bacc
import concourse.bass as bass
import concourse.mybir as mybir
import concourse.tile as tile
from concourse.bass_utils import run_bass_kernel_spmd

DIM = 512
BATCH = 16384
N_CORES = 8
XB = BATCH // N_CORES          # rows per core
P = 128                        # partitions
KT = DIM // P                  # 4 k-tiles
MT = XB // P                   # 16 m-tiles per core
F32 = mybir.dt.float32

_CACHE = {}


def build_bass():
    nc = bacc.Bacc("TRN2", target_bir_lowering=False, debug=False)

    x_d = nc.dram_tensor("x", [XB, DIM], F32, kind="ExternalInput")
    a_d = nc.dram_tensor("a", [DIM, DIM], F32, kind="ExternalInput")
    na_d = nc.dram_tensor("na", [DIM, DIM], F32, kind="ExternalInput")
    bias_d = nc.dram_tensor("bias", [1, DIM], F32, kind="ExternalInput")
    eye_d = nc.dram_tensor("eye", [P, P], F32, kind="ExternalInput")
    out_d = nc.dram_tensor("out", [XB, DIM], F32, kind="ExternalOutput")

    AOP = mybir.AluOpType

    with tile.TileContext(nc) as tc:
        with (
            tc.tile_pool(name="const", bufs=1) as cpool,
            tc.tile_pool(name="xin", bufs=4) as xpool,
            tc.tile_pool(name="xt", bufs=4) as xtpool,
            tc.tile_pool(name="oout", bufs=4) as opool,
            tc.tile_pool(name="eps", bufs=2, space=bass.MemorySpace.PSUM) as eps,
            tc.tile_pool(name="tps", bufs=4, space=bass.MemorySpace.PSUM) as tps,
            tc.tile_pool(name="ops", bufs=2, space=bass.MemorySpace.PSUM) as ops,
        ):
            # ---- replicated small inputs ----
            a_sb = cpool.tile([P, KT, DIM], F32)     # [p, t, n] = A[128t+p, n]
            na_sb = cpool.tile([P, KT, DIM], F32)
            bias_sb = cpool.tile([1, DIM], F32)
            eye_sb = cpool.tile([P, P], F32)
            ones_sb = cpool.tile([1, P], F32)
            for t in range(KT):
                nc.sync.dma_start(a_sb[:, t, :], a_d[P * t : P * (t + 1), :])
                nc.sync.dma_start(na_sb[:, t, :], na_d[P * t : P * (t + 1), :])
            nc.sync.dma_start(bias_sb[:, :], bias_d[:, :])
            nc.sync.dma_start(eye_sb[:, :], eye_d[:, :])
            nc.vector.memset(ones_sb[:, :], 1.0)

            # ---- expm chain (replicated) ----
            a2_sb = cpool.tile([P, KT, DIM], F32)
            a3n_sb = cpool.tile([P, KT, DIM], F32)
            bp_sb = cpool.tile([P, KT, DIM], F32)
            t3_sb = cpool.tile([P, KT, DIM], F32)
            m_sb = cpool.tile([P, KT, DIM], F32)

            # A2 = A @ A   (out rows tile i; contraction tile t)
            for i in range(KT):
                ps = eps.tile([P, DIM], F32, tag="eps")
                for t in range(KT):
                    nc.tensor.matmul(
                        ps[:, :],
                        na_sb[:, t, P * i : P * (i + 1)],
                        a_sb[:, t, :],
                        start=(t == 0),
                        stop=(t == KT - 1),
                    )
                nc.vector.tensor_copy(a2_sb[:, i, :], ps[:, :])

            # A3n = -(A2 @ A) = A2 @ (-A)
            for i in range(KT):
                ps = eps.tile([P, DIM], F32, tag="eps")
                for t in range(KT):
                    nc.tensor.matmul(
                        ps[:, :],
                        a2_sb[:, t, P * i : P * (i + 1)],
                        na_sb[:, t, :],
                        start=(t == 0),
                        stop=(t == KT - 1),
                    )
                nc.vector.tensor_copy(a3n_sb[:, i, :], ps[:, :])

            # B' = A + A2/5 - A3n/30 ; t3 = A + A2/2 - A3n/6
            nc.vector.scalar_tensor_tensor(
                bp_sb[:, :, :], a2_sb[:, :, :], 0.2, a_sb[:, :, :], AOP.mult, AOP.add
            )
            nc.vector.scalar_tensor_tensor(
                bp_sb[:, :, :], a3n_sb[:, :, :], -1.0 / 30.0, bp_sb[:, :, :],
                AOP.mult, AOP.add,
            )
            nc.vector.scalar_tensor_tensor(
                t3_sb[:, :, :], a2_sb[:, :, :], 0.5, a_sb[:, :, :], AOP.mult, AOP.add
            )
            nc.vector.scalar_tensor_tensor(
                t3_sb[:, :, :], a3n_sb[:, :, :], -1.0 / 6.0, t3_sb[:, :, :],
                AOP.mult, AOP.add,
            )

            # F' = A3 @ B' ; M = F'/24 + t3
            for i in range(KT):
                ps = eps.tile([P, DIM], F32, tag="eps")
                for t in range(KT):
                    nc.tensor.matmul(
                        ps[:, :],
                        a3n_sb[:, t, P * i : P * (i + 1)],
                        bp_sb[:, t, :],
                        start=(t == 0),
                        stop=(t == KT - 1),
                    )
                nc.vector.scalar_tensor_tensor(
                    m_sb[:, i, :], ps[:, :], 1.0 / 24.0, t3_sb[:, i, :],
                    AOP.mult, AOP.add,
                )

            # ---- main loop: out = x + x @ M + bias ----
            for mi in range(MT):
                xt = xpool.tile([P, DIM], F32, tag="x")
                nc.sync.dma_start(xt[:, :], x_d[P * mi : P * (mi + 1), :])

                xT = xtpool.tile([P, KT, P], F32, tag="xT")
                for kb in range(KT):
                    tp = tps.tile([P, P], F32, tag="tp")
                    nc.tensor.transpose(
                        tp[:, :], xt[:, P * kb : P * (kb + 1)], eye_sb[:, :]
                    )
                    if kb % 2 == 0:
                        nc.vector.tensor_copy(xT[:, kb, :], tp[:, :])
                    else:
                        nc.scalar.copy(xT[:, kb, :], tp[:, :])

                ps = ops.tile([P, DIM], F32, tag="out")
                for kb in range(KT):
                    nc.tensor.matmul(
                        ps[:, :],
                        xT[:, kb, :],
                        m_sb[:, kb, :],
                        start=(kb == 0),
                        stop=False,
                    )
                # fold bias in as a rank-1 (K=1) matmul: ones^T @ bias
                nc.tensor.matmul(
                    ps[:, :], ones_sb[:, :], bias_sb[:, :], start=False, stop=True
                )

                ot = opool.tile([P, DIM], F32, tag="o")
                nc.vector.tensor_add(ot[:, :], ps[:, :], xt[:, :])
                nc.sync.dma_start(out_d[P * mi : P * (mi + 1), :], ot[:, :])

    nc.compile()
    return nc


def _get_nc():
    if "nc" not in _CACHE:
        _CACHE["nc"] = build_bass()
    return _CACHE["nc"]


def _host_inputs(angles, bias):
    angles = np.asarray(angles, dtype=np.float32)
    bias = np.asarray(bias, dtype=np.float32)
    iu, ju = np.triu_indices(DIM, k=1)
    A = np.zeros((DIM, DIM), dtype=np.float32)
    A[iu, ju] = angles
    A[ju, iu] = -angles
    return {
        "a": A,
        "na": np.ascontiguousarray(-A),
        "bias": bias.reshape(1, DIM),
        "eye": np.eye(P, dtype=np.float32),
    }


def kernel(x, angles, bias, _profile=False):
    x = np.ascontiguousarray(np.asarray(x, dtype=np.float32))
    shared = _host_inputs(angles, bias)
    nc = _get_nc()
    in_maps = [
        {"x": x[XB * c : XB * (c + 1)], **shared} for c in range(N_CORES)
    ]
    res = run_bass_kernel_spmd(
        nc, in_maps, list(range(N_CORES)), trace=bool(_profile)
    )
    _CACHE["last_result"] = res
    out = np.concatenate([res.results[c]["out"] for c in range(N_CORES)], axis=0)
    return out


# revision 9
# speedup vs baseline: 1.2268x; 1.2268x over previous
"""Trainium2 Bass kernel for out = x @ expm(skew(angles)) + bias.

Strategy:
  - Data-parallel over the batch: x [16384, 512] is split into 8 shards of
    [2048, 512], one per NeuronCore. angles/bias are replicated.
  - Host only does layout: builds A = skew(angles) and -A (pure scatter,
    no FLOPs). All linear algebra runs on-device.
  - On each core the rotation is computed via a degree-6 Taylor series in
    Paterson-Stockmeyer form (3 matmuls of 512^3), exploiting skew-symmetry
    so no on-chip transposes of the 512x512 operands are ever needed:
        A2  = A @ A        (lhsT = -A,  since A^T = -A)
        A3n = -A^3         (lhsT = A2,  since A2 is symmetric)
        B'  = A + A2/5 - A3n/30
        F'  = A3 @ B'      (lhsT = A3n, since A3^T = -A3)
        M   = expm(A) - I = A + A2/2 - A3n/6 + F'/24
    The identity never materializes: out = (x + bias) + x @ M.
  - Main matmul: per 128-row tile of x, 4 PE transposes build x^T blocks
    (contraction dim must sit on partitions), then 4 accumulating matmuls
    of N=512.  The final DVE op adds the bias-preloaded residual x tile
    while moving PSUM -> SBUF.
  - Matmul operands are bitcast to float32r: full 4-byte data, but the PE
    streams it at 1 column/cycle (vs 4 cycle-equivalents for plain fp32's
    LOW_HIGH two-pass mode) when the moving free dim is >= 256.

Truncation error of the degree-6 series for this operand norm
(||A||_2 ~ 0.44) is ~4e-8, below fp32 matmul roundoff.
"""

import numpy as np

import concourse.bacc as bacc
import concourse.bass as bass
import concourse.mybir as mybir
import concourse.tile as tile
from concourse.bass_utils import run_bass_kernel_spmd

DIM = 512
BATCH = 16384
N_CORES = 8
XB = BATCH // N_CORES          # rows per core
P = 128                        # partitions
KT = DIM // P                  # 4 k-tiles
MT = XB // P                   # 16 m-tiles per core
F32 = mybir.dt.float32
F32R = mybir.dt.float32r

_CACHE = {}


def build_bass():
    nc = bacc.Bacc("TRN2", target_bir_lowering=False, debug=False)

    x_d = nc.dram_tensor("x", [XB, DIM], F32, kind="ExternalInput")
    a_d = nc.dram_tensor("a", [DIM, DIM], F32, kind="ExternalInput")
    na_d = nc.dram_tensor("na", [DIM, DIM], F32, kind="ExternalInput")
    biasr_d = nc.dram_tensor("biasr", [P, DIM], F32, kind="ExternalInput")
    eye_d = nc.dram_tensor("eye", [P, P], F32, kind="ExternalInput")
    out_d = nc.dram_tensor("out", [XB, DIM], F32, kind="ExternalOutput")

    AOP = mybir.AluOpType

    with tile.TileContext(nc) as tc:
        with (
            tc.tile_pool(name="const", bufs=1) as cpool,
            tc.tile_pool(name="xin", bufs=MT) as xpool,
            tc.tile_pool(name="xt", bufs=MT) as xtpool,
            tc.tile_pool(name="oout", bufs=4) as opool,
            tc.tile_pool(name="eps", bufs=2, space=bass.MemorySpace.PSUM) as eps,
            tc.tile_pool(name="tps", bufs=4, space=bass.MemorySpace.PSUM) as tps,
            tc.tile_pool(name="ops", bufs=2, space=bass.MemorySpace.PSUM) as ops,
        ):
            # ---- input loads: x tiles + eye first so PE can start early ----
            eye_sb = cpool.tile([P, P], F32)
            nc.sync.dma_start(eye_sb[:, :], eye_d[:, :])
            xts = []
            for mi in range(MT):
                xt = xpool.tile([P, DIM], F32, tag="x")
                nc.sync.dma_start(xt[:, :], x_d[P * mi : P * (mi + 1), :])
                xts.append(xt)

            a_sb = cpool.tile([P, KT, DIM], F32)     # [p, t, n] = A[128t+p, n]
            na_sb = cpool.tile([P, KT, DIM], F32)
            biasr_sb = cpool.tile([P, DIM], F32)
            for t in range(KT):
                nc.sync.dma_start(a_sb[:, t, :], a_d[P * t : P * (t + 1), :])
                nc.sync.dma_start(na_sb[:, t, :], na_d[P * t : P * (t + 1), :])
            nc.sync.dma_start(biasr_sb[:, :], biasr_d[:, :])

            # ---- transposes: xT blocks (contraction on partitions) ----
            xTs = []
            for mi in range(MT):
                xT = xtpool.tile([P, KT, P], F32, tag="xT")
                for kb in range(KT):
                    tp = tps.tile([P, P], F32, tag="tp")
                    nc.tensor.transpose(
                        tp[:, :], xts[mi][:, P * kb : P * (kb + 1)], eye_sb[:, :]
                    )
                    if kb % 2 == 0:
                        nc.vector.tensor_copy(xT[:, kb, :], tp[:, :])
                    else:
                        nc.scalar.copy(xT[:, kb, :], tp[:, :])
                xTs.append(xT)
                # residual + bias folded into the x tile in place (after the
                # transposes have consumed it)
                nc.vector.tensor_add(xts[mi][:, :], xts[mi][:, :], biasr_sb[:, :])

            # ---- expm chain (replicated) ----
            # Matmul operands are float32r (4-byte fp32 rounded to 11
            # mantissa bits): the PE streams fp32r at 1 column/cycle vs 4
            # cycle-equivalents for plain fp32's two-pass LOW_HIGH mode.
            # Operand magnitudes here are ~1e-2, so the 2^-12 input rounding
            # contributes only ~5e-6 to the final output — far below the
            # fp32 matmul roundoff of the main product.  The main x@M
            # matmul stays full fp32.  DVE writes into fp32r tiles perform
            # the rounding the BIR verifier requires of fp32r producers.
            ar_sb = cpool.tile([P, KT, DIM], F32R)
            nar_sb = cpool.tile([P, KT, DIM], F32R)
            nc.vector.tensor_copy(ar_sb[:, :, :], a_sb[:, :, :])
            nc.vector.tensor_copy(nar_sb[:, :, :], na_sb[:, :, :])

            a2_sb = cpool.tile([P, KT, DIM], F32R)
            a3n_sb = cpool.tile([P, KT, DIM], F32R)
            bp_sb = cpool.tile([P, KT, DIM], F32R)
            t3_sb = cpool.tile([P, KT, DIM], F32)
            m_sb = cpool.tile([P, KT, DIM], F32)

            # A2 = A @ A
            for i in range(KT):
                ps = eps.tile([P, DIM], F32, tag="eps")
                for t in range(KT):
                    nc.tensor.matmul(
                        ps[:, :],
                        nar_sb[:, t, P * i : P * (i + 1)],
                        ar_sb[:, t, :],
                        start=(t == 0),
                        stop=(t == KT - 1),
                    )
                nc.vector.tensor_copy(a2_sb[:, i, :], ps[:, :])

            # A3n = -(A2 @ A) = A2 @ (-A)
            for i in range(KT):
                ps = eps.tile([P, DIM], F32, tag="eps")
                for t in range(KT):
                    nc.tensor.matmul(
                        ps[:, :],
                        a2_sb[:, t, P * i : P * (i + 1)],
                        nar_sb[:, t, :],
                        start=(t == 0),
                        stop=(t == KT - 1),
                    )
                nc.vector.tensor_copy(a3n_sb[:, i, :], ps[:, :])

            # B' = A + A2/5 - A3n/30 ; t3 = A + A2/2 - A3n/6
            nc.vector.scalar_tensor_tensor(
                bp_sb[:, :, :], a2_sb[:, :, :], 0.2, a_sb[:, :, :], AOP.mult, AOP.add
            )
            nc.vector.scalar_tensor_tensor(
                bp_sb[:, :, :], a3n_sb[:, :, :], -1.0 / 30.0, bp_sb[:, :, :],
                AOP.mult, AOP.add,
            )
            nc.vector.scalar_tensor_tensor(
                t3_sb[:, :, :], a2_sb[:, :, :], 0.5, a_sb[:, :, :], AOP.mult, AOP.add
            )
            nc.vector.scalar_tensor_tensor(
                t3_sb[:, :, :], a3n_sb[:, :, :], -1.0 / 6.0, t3_sb[:, :, :],
                AOP.mult, AOP.add,
            )

            # F' = A3 @ B' ; M = F'/24 + t3
            for i in range(KT):
                ps = eps.tile([P, DIM], F32, tag="eps")
                for t in range(KT):
                    nc.tensor.matmul(
                        ps[:, :],
                        a3n_sb[:, t, P * i : P * (i + 1)],
                        bp_sb[:, t, :],
                        start=(t == 0),
                        stop=(t == KT - 1),
                    )
                nc.vector.scalar_tensor_tensor(
                    m_sb[:, i, :], ps[:, :], 1.0 / 24.0, t3_sb[:, i, :],
                    AOP.mult, AOP.add,
                )

            # ---- main loop: out = (x + bias) + x @ M ----
            for mi in range(MT):
                ps = ops.tile([P, DIM], F32, tag="out")
                for kb in range(KT):
                    nc.tensor.matmul(
                        ps[:, :],
                        xTs[mi][:, kb, :],
                        m_sb[:, kb, :],
                        start=(kb == 0),
                        stop=(kb == KT - 1),
                    )
                ot = opool.tile([P, DIM], F32, tag="o")
                nc.vector.tensor_add(ot[:, :], ps[:, :], xts[mi][:, :])
                nc.sync.dma_start(out_d[P * mi : P * (mi + 1), :], ot[:, :])

    nc.compile()
    return nc


def _get_nc():
    if "nc" not in _CACHE:
        _CACHE["nc"] = build_bass()
    return _CACHE["nc"]


def _host_inputs(angles, bias):
    angles = np.asarray(angles, dtype=np.float32)
    bias = np.asarray(bias, dtype=np.float32)
    iu, ju = np.triu_indices(DIM, k=1)
    A = np.zeros((DIM, DIM), dtype=np.float32)
    A[iu, ju] = angles
    A[ju, iu] = -angles
    return {
        "a": A,
        "na": np.ascontiguousarray(-A),
        "biasr": np.ascontiguousarray(
            np.broadcast_to(bias.reshape(1, DIM), (P, DIM))
        ),
        "eye": np.eye(P, dtype=np.float32),
    }


def kernel(x, angles, bias, _profile=False):
    x = np.ascontiguousarray(np.asarray(x, dtype=np.float32))
    shared = _host_inputs(angles, bias)
    nc = _get_nc()
    in_maps = [
        {"x": x[XB * c : XB * (c + 1)], **shared} for c in range(N_CORES)
    ]
    res = run_bass_kernel_spmd(
        nc, in_maps, list(range(N_CORES)), trace=bool(_profile)
    )
    _CACHE["last_result"] = res
    out = np.concatenate([res.results[c]["out"] for c in range(N_CORES)], axis=0)
    return out


# revision 10
# speedup vs baseline: 1.4171x; 1.1551x over previous
"""Trainium2 Bass kernel for out = x @ expm(skew(angles)) + bias.

Strategy:
  - Data-parallel over the batch: x [16384, 512] is split into 8 shards of
    [2048, 512], one per NeuronCore. angles/bias are replicated.
  - Host only does layout: builds A = skew(angles), -A and the fp32r
    roundings of both (pure scatter/bit ops, no FLOPs). All linear algebra
    runs on-device.
  - On each core the rotation is computed via a degree-6 Taylor series in
    Paterson-Stockmeyer form (3 matmuls of 512^3), exploiting skew-symmetry
    so no on-chip transposes of the 512x512 operands are ever needed:
        A2  = A @ A        (lhsT = -A,  since A^T = -A)
        A3n = -A^3         (lhsT = A2,  since A2 is symmetric)
        B'  = A + A2/5 - A3n/30
        F'  = A3 @ B'      (lhsT = A3n, since A3^T = -A3)
        M   = expm(A) - I = A + A2/2 - A3n/6 + F'/24
    The identity never materializes: out = (x + bias) + x @ M.
  - expm matmul operands are float32r (fp32 rounded to 11 mantissa bits):
    the PE streams fp32r at 1 column/cycle vs 4 cycle-equivalents for plain
    fp32's two-pass LOW_HIGH mode.  Operand magnitudes there are ~1e-2, so
    the 2^-12 input rounding contributes only ~2e-5 absolute to the output.
    The main x@M matmul stays full fp32 for precision; the linear terms of
    M are built from the exact (unrounded) A.
  - Main matmul: per 128-row tile of x, 4 PE transposes build x^T blocks
    (contraction dim must sit on partitions), then 4 accumulating fp32
    matmuls of N=512.  The final DVE op adds the bias-preloaded residual
    x tile while moving PSUM -> SBUF.
  - Program order puts the expm chain (the critical path to M) ahead of
    the transposes so the Tile scheduler fills expm's DVE-stall gaps with
    transpose work.
"""

import numpy as np

import concourse.bacc as bacc
import concourse.bass as bass
import concourse.mybir as mybir
import concourse.tile as tile
from concourse.bass_utils import run_bass_kernel_spmd

DIM = 512
BATCH = 16384
N_CORES = 8
XB = BATCH // N_CORES          # rows per core
P = 128                        # partitions
KT = DIM // P                  # 4 k-tiles
MT = XB // P                   # 16 m-tiles per core
F32 = mybir.dt.float32
F32R = mybir.dt.float32r

_CACHE = {}


def build_bass():
    nc = bacc.Bacc("TRN2", target_bir_lowering=False, debug=False)

    x_d = nc.dram_tensor("x", [XB, DIM], F32, kind="ExternalInput")
    a_d = nc.dram_tensor("a", [DIM, DIM], F32, kind="ExternalInput")
    ar_d = nc.dram_tensor("ar", [DIM, DIM], F32R, kind="ExternalInput")
    nar_d = nc.dram_tensor("nar", [DIM, DIM], F32R, kind="ExternalInput")
    biasr_d = nc.dram_tensor("biasr", [P, DIM], F32, kind="ExternalInput")
    eye_d = nc.dram_tensor("eye", [P, P], F32, kind="ExternalInput")
    out_d = nc.dram_tensor("out", [XB, DIM], F32, kind="ExternalOutput")

    AOP = mybir.AluOpType

    with tile.TileContext(nc) as tc:
        with (
            tc.tile_pool(name="const", bufs=1) as cpool,
            tc.tile_pool(name="xin", bufs=MT) as xpool,
            tc.tile_pool(name="xt", bufs=MT) as xtpool,
            tc.tile_pool(name="oout", bufs=4) as opool,
            tc.tile_pool(name="eps", bufs=2, space=bass.MemorySpace.PSUM) as eps,
            tc.tile_pool(name="tps", bufs=4, space=bass.MemorySpace.PSUM) as tps,
            tc.tile_pool(name="ops", bufs=2, space=bass.MemorySpace.PSUM) as ops,
        ):
            # ---- small replicated inputs first (expm critical path) ----
            a_sb = cpool.tile([P, KT, DIM], F32)     # [p, t, n] = A[128t+p, n]
            ar_sb = cpool.tile([P, KT, DIM], F32R)
            nar_sb = cpool.tile([P, KT, DIM], F32R)
            biasr_sb = cpool.tile([P, DIM], F32)
            eye_sb = cpool.tile([P, P], F32)
            for t in range(KT):
                nc.sync.dma_start(ar_sb[:, t, :], ar_d[P * t : P * (t + 1), :])
                nc.sync.dma_start(nar_sb[:, t, :], nar_d[P * t : P * (t + 1), :])
                nc.sync.dma_start(a_sb[:, t, :], a_d[P * t : P * (t + 1), :])
            nc.sync.dma_start(eye_sb[:, :], eye_d[:, :])
            nc.sync.dma_start(biasr_sb[:, :], biasr_d[:, :])

            # ---- x tile loads ----
            xts = []
            for mi in range(MT):
                xt = xpool.tile([P, DIM], F32, tag="x")
                nc.sync.dma_start(xt[:, :], x_d[P * mi : P * (mi + 1), :])
                xts.append(xt)

            # ---- expm chain (replicated; fp32r operands) ----
            a2_sb = cpool.tile([P, KT, DIM], F32R)
            a3n_sb = cpool.tile([P, KT, DIM], F32R)
            bp_sb = cpool.tile([P, KT, DIM], F32R)
            t3_sb = cpool.tile([P, KT, DIM], F32)
            m_sb = cpool.tile([P, KT, DIM], F32)

            # A2 = A @ A
            for i in range(KT):
                ps = eps.tile([P, DIM], F32, tag="eps")
                for t in range(KT):
                    nc.tensor.matmul(
                        ps[:, :],
                        nar_sb[:, t, P * i : P * (i + 1)],
                        ar_sb[:, t, :],
                        start=(t == 0),
                        stop=(t == KT - 1),
                    )
                # split the copy across DVE and ACT to shorten the serial
                # barrier before the next matmul group
                if i % 2 == 0:
                    nc.vector.tensor_copy(a2_sb[:, i, :], ps[:, :])
                else:
                    nc.scalar.copy(a2_sb[:, i, :], ps[:, :])

            # A3n = -(A2 @ A) = A2 @ (-A)
            for i in range(KT):
                ps = eps.tile([P, DIM], F32, tag="eps")
                for t in range(KT):
                    nc.tensor.matmul(
                        ps[:, :],
                        a2_sb[:, t, P * i : P * (i + 1)],
                        nar_sb[:, t, :],
                        start=(t == 0),
                        stop=(t == KT - 1),
                    )
                if i % 2 == 0:
                    nc.vector.tensor_copy(a3n_sb[:, i, :], ps[:, :])
                else:
                    nc.scalar.copy(a3n_sb[:, i, :], ps[:, :])

            # B' = A + A2/5 - A3n/30 ; t3 = A + A2/2 - A3n/6
            nc.vector.scalar_tensor_tensor(
                bp_sb[:, :, :], a2_sb[:, :, :], 0.2, a_sb[:, :, :], AOP.mult, AOP.add
            )
            nc.vector.scalar_tensor_tensor(
                bp_sb[:, :, :], a3n_sb[:, :, :], -1.0 / 30.0, bp_sb[:, :, :],
                AOP.mult, AOP.add,
            )
            nc.vector.scalar_tensor_tensor(
                t3_sb[:, :, :], a2_sb[:, :, :], 0.5, a_sb[:, :, :], AOP.mult, AOP.add
            )
            nc.vector.scalar_tensor_tensor(
                t3_sb[:, :, :], a3n_sb[:, :, :], -1.0 / 6.0, t3_sb[:, :, :],
                AOP.mult, AOP.add,
            )

            # F' = A3 @ B' ; M = F'/24 + t3
            for i in range(KT):
                ps = eps.tile([P, DIM], F32, tag="eps")
                for t in range(KT):
                    nc.tensor.matmul(
                        ps[:, :],
                        a3n_sb[:, t, P * i : P * (i + 1)],
                        bp_sb[:, t, :],
                        start=(t == 0),
                        stop=(t == KT - 1),
                    )
                nc.vector.scalar_tensor_tensor(
                    m_sb[:, i, :], ps[:, :], 1.0 / 24.0, t3_sb[:, i, :],
                    AOP.mult, AOP.add,
                )

            # ---- transposes: xT blocks (fill PE gaps in the expm chain) ----
            xTs = []
            for mi in range(MT):
                xT = xtpool.tile([P, KT, P], F32, tag="xT")
                for kb in range(KT):
                    tp = tps.tile([P, P], F32, tag="tp")
                    nc.tensor.transpose(
                        tp[:, :], xts[mi][:, P * kb : P * (kb + 1)], eye_sb[:, :]
                    )
                    if kb % 2 == 0:
                        nc.vector.tensor_copy(xT[:, kb, :], tp[:, :])
                    else:
                        nc.scalar.copy(xT[:, kb, :], tp[:, :])
                xTs.append(xT)
                # residual + bias folded into the x tile in place (after the
                # transposes have consumed it)
                nc.vector.tensor_add(xts[mi][:, :], xts[mi][:, :], biasr_sb[:, :])

            # ---- main loop: out = (x + bias) + x @ M ----
            for mi in range(MT):
                ps = ops.tile([P, DIM], F32, tag="out")
                for kb in range(KT):
                    nc.tensor.matmul(
                        ps[:, :],
                        xTs[mi][:, kb, :],
                        m_sb[:, kb, :],
                        start=(kb == 0),
                        stop=(kb == KT - 1),
                    )
                ot = opool.tile([P, DIM], F32, tag="o")
                nc.vector.tensor_add(ot[:, :], ps[:, :], xts[mi][:, :])
                nc.sync.dma_start(out_d[P * mi : P * (mi + 1), :], ot[:, :])

    nc.compile()
    return nc


def _get_nc():
    if "nc" not in _CACHE:
        _CACHE["nc"] = build_bass()
    return _CACHE["nc"]


def _round_fp32r(x):
    """Round-to-nearest-even to 11 mantissa bits (verified bit-exact
    against walrus's fp32_to_fp32r)."""
    b = np.ascontiguousarray(x, dtype=np.float32).view(np.uint32).astype(np.uint64)
    b = b + 0x7FF + ((b >> 12) & 1)
    return (b & np.uint64(0xFFFFF000)).astype(np.uint32).view(np.float32)


def _host_inputs(angles, bias):
    angles = np.asarray(angles, dtype=np.float32)
    bias = np.asarray(bias, dtype=np.float32)
    iu, ju = np.triu_indices(DIM, k=1)
    A = np.zeros((DIM, DIM), dtype=np.float32)
    A[iu, ju] = angles
    A[ju, iu] = -angles
    return {
        "a": A,
        "ar": _round_fp32r(A),
        "nar": _round_fp32r(-A),
        "biasr": np.ascontiguousarray(
            np.broadcast_to(bias.reshape(1, DIM), (P, DIM))
        ),
        "eye": np.eye(P, dtype=np.float32),
    }


def kernel(x, angles, bias, _profile=False):
    x = np.ascontiguousarray(np.asarray(x, dtype=np.float32))
    shared = _host_inputs(angles, bias)
    nc = _get_nc()
    in_maps = [
        {"x": x[XB * c : XB * (c + 1)], **shared} for c in range(N_CORES)
    ]
    res = run_bass_kernel_spmd(
        nc, in_maps, list(range(N_CORES)), trace=bool(_profile)
    )
    _CACHE["last_result"] = res
    out = np.concatenate([res.results[c]["out"] for c in range(N_CORES)], axis=0)
    return out
